# revision 15
# baseline (speedup 1.0000x reference)
"""Trainium2 Bass kernel for the gnn_message_passing problem.

Strategy (8 NeuronCores, SPMD), v2:
  - Host computes U = unique(inputs|item) (~32k of 50k vocab rows actually
    referenced).  Each core projects a 4096-row slice of U to the 128-dim
    item embedding (emb + 0.1*(img@Wi+bi) + 0.15*(txt@Wt+bt)) using
    weights-stationary N=512 bf16 matmuls in transposed orientation
    ([dout, rows]), then PE-transposes back to row-major bf16.
  - ONE bf16 AllGather of the compact item table (8.4MB vs 51MB in v1).
    The Tvis (img|txt projected) all-gather is eliminated entirely via
    linearity: session_img = (W^T @ masked_row_sum + b*cnt)/denom, so the
    session means are computed from RAW table rows gathered per batch
    shard from a U-compacted [32769, 1896] bf16 concat table, with
    mask-stationary [100,2]x[100,512] matmuls (few, large instructions).
  - Batch-sharded phase C: session fusion math in transposed [128, 64]
    layout (as v1), hypergraph layers per session pair in bf16.
"""

import sys

sys.path.insert(0, "/opt/trn_rl_repo")

import numpy as np
import ml_dtypes

import concourse.bass as bass
import concourse.bacc as bacc
import concourse.mybir as mybir
import concourse.tile as tile
from concourse import bass_utils

BF16 = ml_dtypes.bfloat16


class Cfg:
    def __init__(self):
        self.N = 50000
        self.D = 128
        self.IMG = 1000
        self.TXT = 768
        self.B = 512
        self.L = 50
        self.NC = 8
        self.UC = 4096                 # U rows projected per core
        self.NU = self.NC * self.UC    # 32768 capacity
        self.NF = 1 + self.NU          # padded table rows (row 0 = zeros)
        self.BS = self.B // self.NC    # 64 sessions per core
        self.NPAIR = self.BS // 2      # 32
        self.L2 = 2 * self.L           # 100
        self.KI = 8                    # img k-chunks of 125
        self.KIW = 125
        self.KT = 6                    # txt k-chunks of 128
        self.KTW = 128
        self.RAWW = self.IMG + self.TXT + self.D   # 1896
        self.ACH = self.UC // 512      # 8 phase-A chunks of 512 rows


REAL = Cfg()


def build_program(c: Cfg):
    f32 = mybir.dt.float32
    b16 = mybir.dt.bfloat16
    i32 = mybir.dt.int32
    AF = mybir.ActivationFunctionType
    AX = mybir.AxisListType
    OP = mybir.AluOpType

    nc = bacc.Bacc("TRN2", target_bir_lowering=False, debug=False,
                   num_devices=c.NC)

    def ein(nm, sh, dt):
        return nc.dram_tensor(nm, sh, dt, kind="ExternalInput")

    imgRT = ein("imgRT", [c.IMG, c.UC], b16)    # img rows of U_c, transposed
    txtRT = ein("txtRT", [c.TXT, c.UC], b16)
    embRT = ein("embRT", [c.D, c.UC], f32)      # emb rows of U_c, transposed
    wiN = ein("wiN", [c.IMG, c.D], b16)         # img_W
    wtN = ein("wtN", [c.TXT, c.D], b16)
    wi01 = ein("wi01", [c.IMG, c.D], b16)       # 0.1 * img_W
    wt015 = ein("wt015", [c.TXT, c.D], b16)     # 0.15 * txt_W
    imgbR = ein("imgbR", [1, c.D], b16)
    txtbR = ein("txtbR", [1, c.D], b16)
    rawcat = ein("rawcat", [c.NF, c.RAWW], b16)  # U-compacted img|txt|emb
    gvW = ein("gvW", [c.D, c.D], f32)
    gvB = ein("gvB", [c.D, 1], f32)
    gtW = ein("gtW", [c.D, c.D], f32)
    gtB = ein("gtB", [c.D, 1], f32)
    q1W = ein("q1W", [c.D, c.D], f32)
    q1B = ein("q1B", [c.D, 1], f32)
    q2W = ein("q2W", [c.D, 1], f32)
    Gbd = ein("Gbd", [c.NPAIR, c.L2, c.L2], b16)
    GTbd = ein("GTbd", [c.NPAIR, c.L2, c.L2], b16)
    Mbd16 = ein("Mbd16", [c.NPAIR, c.L2, 2], b16)
    mkT = ein("mkT", [c.L, c.BS], b16)          # mask.T
    mindT = ein("mindT", [c.L, c.BS], b16)      # (mask * (item>0)).T
    ind2 = ein("ind2", [2, c.L2], b16)
    h0idx = ein("h0idx", [c.NPAIR, c.L2, 1], i32)
    ssidx = ein("ssidx", [c.NPAIR, c.L2, 1], i32)

    outH = nc.dram_tensor("outH", [c.BS, c.L, c.D], f32, kind="ExternalOutput")

    localI = nc.dram_tensor("localI", [c.UC, c.D], b16)
    Titem = nc.dram_tensor("Titem", [c.NF, c.D], b16, addr_space="Shared")

    rg = [list(range(c.NC))]
    # raw-concat column chunks for transposes / projection
    # img: 8 x 125, txt: 6 x 128, emb: 1 x 128
    CH = [(k * c.KIW, c.KIW) for k in range(c.KI)]
    CH += [(c.IMG + k * c.KTW, c.KTW) for k in range(c.KT)]
    CH += [(c.IMG + c.TXT, c.D)]
    # session-sum segments (psum free-dim <= 512)
    SEG = [(0, 512), (512, 512), (1024, 512), (1536, 360)]

    with tile.TileContext(nc) as tc:
        with (
            tc.tile_pool(name="wpool", bufs=1) as wp,
            tc.tile_pool(name="apool", bufs=2) as ap,
            tc.tile_pool(name="ostg", bufs=2) as ost,
            tc.tile_pool(name="cbig", bufs=1) as cb,
            tc.tile_pool(name="cgat", bufs=4) as cg,
            tc.tile_pool(name="csml", bufs=3) as cs,
        ):
            # ---- weights / constants ----
            wi01t = [wp.tile([c.KIW, c.D], b16, tag=f"wi01_{k}", name=f"wi01_{k}")
                     for k in range(c.KI)]
            wt015t = [wp.tile([c.KTW, c.D], b16, tag=f"wt015_{k}", name=f"wt015_{k}")
                      for k in range(c.KT)]
            wiNt = [wp.tile([c.KIW, c.D], b16, tag=f"wiN_{k}", name=f"wiN_{k}")
                    for k in range(c.KI)]
            wtNt = [wp.tile([c.KTW, c.D], b16, tag=f"wtN_{k}", name=f"wtN_{k}")
                    for k in range(c.KT)]
            for k in range(c.KI):
                nc.sync.dma_start(wi01t[k][:], wi01[k * c.KIW:(k + 1) * c.KIW, :])
                nc.sync.dma_start(wiNt[k][:], wiN[k * c.KIW:(k + 1) * c.KIW, :])
            for k in range(c.KT):
                nc.sync.dma_start(wt015t[k][:], wt015[k * c.KTW:(k + 1) * c.KTW, :])
                nc.sync.dma_start(wtNt[k][:], wtN[k * c.KTW:(k + 1) * c.KTW, :])
            bir_ = wp.tile([1, c.D], b16, tag="bir")
            btr = wp.tile([1, c.D], b16, tag="btr")
            nc.sync.dma_start(bir_[:], imgbR[:])
            nc.sync.dma_start(btr[:], txtbR[:])
            ident16 = wp.tile([128, 128], b16, tag="id16")
            identf = wp.tile([128, 128], f32, tag="idf")
            from concourse.masks import make_identity
            make_identity(nc, ident16[:])
            make_identity(nc, identf[:])

            # zero row 0 of Titem
            zi = wp.tile([1, c.D], b16, tag="zi")
            nc.vector.memset(zi[:], 0.0)
            nc.sync.dma_start(Titem[0:1, :], zi[:])

            # phase-C persistent loads (start DMAs early; no dep on phase A)
            m16 = cb.tile([c.L2, c.NPAIR * 2], b16, tag="m16")
            nc.sync.dma_start(
                m16[:].rearrange("l (p j) -> l p j", p=c.NPAIR),
                Mbd16.rearrange("p l j -> l p j"))
            Gsb = cb.tile([c.L2, c.NPAIR * c.L2], b16, tag="Gsb")
            GTsb = cb.tile([c.L2, c.NPAIR * c.L2], b16, tag="GTsb")
            nc.sync.dma_start(
                Gsb[:].rearrange("l (p e) -> l p e", p=c.NPAIR),
                Gbd.rearrange("p l e -> l p e"))
            nc.sync.dma_start(
                GTsb[:].rearrange("l (p e) -> l p e", p=c.NPAIR),
                GTbd.rearrange("p l e -> l p e"))
            hix = cb.tile([c.L2, c.NPAIR], i32, tag="hix")
            six = cb.tile([c.L2, c.NPAIR], i32, tag="six")
            nc.sync.dma_start(hix[:], h0idx.rearrange("p l o -> l (p o)"))
            nc.sync.dma_start(six[:], ssidx.rearrange("p l o -> l (p o)"))
            mkTt = cb.tile([c.L, c.BS], b16, tag="mkT")
            minTt = cb.tile([c.L, c.BS], b16, tag="minT")
            nc.sync.dma_start(mkTt[:], mkT[:])
            nc.sync.dma_start(minTt[:], mindT[:])
            i2 = cb.tile([2, c.L2], b16, tag="i2")
            nc.sync.dma_start(i2[:], ind2[:])
            ones50 = cb.tile([c.L, 1], b16, tag="ones50")
            nc.vector.memset(ones50[:], 1.0)
            wgv = cb.tile([c.D, c.D], f32, tag="wgv")
            wgt = cb.tile([c.D, c.D], f32, tag="wgt")
            wq1 = cb.tile([c.D, c.D], f32, tag="wq1")
            wq2 = cb.tile([c.D, 1], f32, tag="wq2")
            bgv = cb.tile([c.D, 1], f32, tag="bgv")
            bgt = cb.tile([c.D, 1], f32, tag="bgt")
            bq1 = cb.tile([c.D, 1], f32, tag="bq1")
            nc.sync.dma_start(wgv[:], gvW[:])
            nc.sync.dma_start(wgt[:], gtW[:])
            nc.sync.dma_start(wq1[:], q1W[:])
            nc.sync.dma_start(wq2[:], q2W[:])
            nc.sync.dma_start(bgv[:], gvB[:])
            nc.sync.dma_start(bgt[:], gtB[:])
            nc.sync.dma_start(bq1[:], q1B[:])

            psq_ctx = tc.tile_pool(name="psq", bufs=1, space="PSUM")
            psq = psq_ctx.__enter__()
            # denom / cnt row vectors
            dT = psq.tile([1, c.BS], f32, tag="q0", name="dT")
            nc.tensor.matmul(dT[:], lhsT=ones50[:], rhs=mkTt[:],
                             start=True, stop=True)
            invd = cb.tile([1, c.BS], f32, tag="invd")
            nc.vector.reciprocal(invd[:], dT[:])
            cT = psq.tile([1, c.BS], f32, tag="q0", name="cT")
            nc.tensor.matmul(cT[:], lhsT=ones50[:], rhs=minTt[:],
                             start=True, stop=True)
            cntR = cb.tile([1, c.BS], b16, tag="cntR")
            nc.vector.tensor_copy(cntR[:], cT[:])

            # ================= Phase A: project U_c rows =================
            imgRT3 = imgRT.rearrange("(k f) v -> f k v", k=c.KI)
            txtRT3 = txtRT.rearrange("(k f) v -> f k v", k=c.KT)
            psA_ctx = tc.tile_pool(name="psA", bufs=2, space="PSUM")
            psA = psA_ctx.__enter__()
            psB_ctx = tc.tile_pool(name="psB", bufs=2, space="PSUM")
            psB = psB_ctx.__enter__()
            for cc in range(c.ACH):
                v0 = cc * 512
                ai = ap.tile([c.KIW, c.KI * 512], b16, tag="ai")
                nc.sync.dma_start(
                    ai[:].rearrange("f (k v) -> f k v", k=c.KI),
                    imgRT3[:, :, v0:v0 + 512])
                at = ap.tile([c.KTW, c.KT * 512], b16, tag="at")
                nc.sync.dma_start(
                    at[:].rearrange("f (k v) -> f k v", k=c.KT),
                    txtRT3[:, :, v0:v0 + 512])
                ae = ap.tile([c.D, 512], f32, tag="ae")
                nc.sync.dma_start(ae[:], embRT[:, v0:v0 + 512])
                pst = psA.tile([c.D, 512], f32, tag="psA")
                for k in range(c.KI):
                    nc.tensor.matmul(pst[:], lhsT=wi01t[k][:],
                                     rhs=ai[:, k * 512:(k + 1) * 512],
                                     start=(k == 0), stop=False)
                for k in range(c.KT):
                    nc.tensor.matmul(pst[:], lhsT=wt015t[k][:],
                                     rhs=at[:, k * 512:(k + 1) * 512],
                                     start=False, stop=(k == c.KT - 1))
                io = ost.tile([c.D, 512], f32, tag="io")
                nc.vector.tensor_add(io[:], pst[:], ae[:])
                st = ost.tile([128, 512], b16, tag="st")
                for j in range(4):
                    trj = psB.tile([128, 128], f32, tag="trj")
                    nc.tensor.transpose(trj[:], io[:, j * 128:(j + 1) * 128],
                                        identf[:])
                    if j % 2 == 0:
                        nc.scalar.copy(st[:, j * 128:(j + 1) * 128], trj[:])
                    else:
                        nc.vector.tensor_copy(st[:, j * 128:(j + 1) * 128], trj[:])
                nc.sync.dma_start(
                    localI[v0:v0 + 512, :].rearrange("(n p) d -> p n d", n=4),
                    st[:].rearrange("p (n d) -> p n d", n=4))

            psB_ctx.__exit__(None, None, None)
            psA_ctx.__exit__(None, None, None)

            # ================= AllGather (bf16 item table) =================
            nc.gpsimd.collective_compute(
                "AllGather", mybir.AluOpType.bypass, replica_groups=rg,
                ins=[localI[:].opt()], outs=[Titem[1:c.NF, :].opt()])

            # h0 gathers (depend on AG; DMA overlaps session sums below)
            h0all = cb.tile([c.L2, c.NPAIR * c.D], b16, tag="h0all")
            for p in range(c.NPAIR):
                nc.gpsimd.indirect_dma_start(
                    out=h0all[:, p * c.D:(p + 1) * c.D], out_offset=None,
                    in_=Titem[:],
                    in_offset=bass.IndirectOffsetOnAxis(
                        ap=hix[:, p:p + 1], axis=0))

            # ========== session raw sums (independent of phase A / AG) =====
            Sraw = cb.tile([c.BS, c.RAWW], f32, tag="Sraw")
            psS_ctx = tc.tile_pool(name="psS", bufs=2, space="PSUM")
            psS = psS_ctx.__enter__()
            for p in range(c.NPAIR):
                gvi = cg.tile([c.L2, c.RAWW], b16, tag="gvi")
                nc.gpsimd.indirect_dma_start(
                    out=gvi[:], out_offset=None, in_=rawcat[:],
                    in_offset=bass.IndirectOffsetOnAxis(
                        ap=six[:, p:p + 1], axis=0))
                stg = cs.tile([2, c.RAWW], f32, tag="stg")
                for jj, (s0, w) in enumerate(SEG):
                    pss = psS.tile([2, 512], f32, tag="pss")
                    nc.tensor.matmul(pss[:, 0:w], lhsT=m16[:, 2 * p:2 * p + 2],
                                     rhs=gvi[:, s0:s0 + w],
                                     start=True, stop=True)
                    if jj % 2 == 0:
                        nc.scalar.copy(stg[:, s0:s0 + w], pss[:, 0:w])
                    else:
                        nc.vector.tensor_copy(stg[:, s0:s0 + w], pss[:, 0:w])
                nc.sync.dma_start(Sraw[2 * p:2 * p + 2, :], stg[:])

            # transpose Sraw -> SrawT chunks [d-chunk, 64] bf16
            psT_ctx = tc.tile_pool(name="psT", bufs=2, space="PSUM")
            psT = psT_ctx.__enter__()
            SrawT = cb.tile([128, len(CH) * c.BS], b16, tag="SrawT")
            for ch, (s0, w) in enumerate(CH):
                trS = psT.tile([128, c.BS], f32, tag="trS")
                nc.tensor.transpose(trS[0:w, :], Sraw[:, s0:s0 + w],
                                    identf[0:c.BS, 0:c.BS])
                nc.scalar.copy(SrawT[0:w, ch * c.BS:(ch + 1) * c.BS],
                               trS[0:w, :])
            psT_ctx.__exit__(None, None, None)
            psS_ctx.__exit__(None, None, None)

            # ========== project session sums:  X = W^T S + b*cnt ==========
            psg_ctx = tc.tile_pool(name="psg", bufs=2, space="PSUM")
            psg = psg_ctx.__enter__()
            XimP = psg.tile([c.D, c.BS], f32, tag="px", name="XimP")
            for k in range(c.KI):
                nc.tensor.matmul(XimP[:], lhsT=wiNt[k][:],
                                 rhs=SrawT[0:c.KIW, k * c.BS:(k + 1) * c.BS],
                                 start=(k == 0), stop=False)
            nc.tensor.matmul(XimP[:], lhsT=bir_[:], rhs=cntR[:],
                             start=False, stop=True)
            Xim = cb.tile([c.D, c.BS], f32, tag="Xim")
            nc.scalar.copy(Xim[:], XimP[:])
            XtxP = psg.tile([c.D, c.BS], f32, tag="px", name="XtxP")
            for k in range(c.KT):
                nc.tensor.matmul(
                    XtxP[:], lhsT=wtNt[k][:],
                    rhs=SrawT[0:c.KTW, (c.KI + k) * c.BS:(c.KI + k + 1) * c.BS],
                    start=(k == 0), stop=False)
            nc.tensor.matmul(XtxP[:], lhsT=btr[:], rhs=cntR[:],
                             start=False, stop=True)
            Xtx = cb.tile([c.D, c.BS], f32, tag="Xtx")
            nc.scalar.copy(Xtx[:], XtxP[:])
            # Xit = Semb + 0.1 Xim + 0.15 Xtx
            sembc = cs.tile([c.D, c.BS], f32, tag="sembc")
            nc.scalar.copy(sembc[:],
                           SrawT[:, (c.KI + c.KT) * c.BS:(c.KI + c.KT + 1) * c.BS])
            t1 = cs.tile([c.D, c.BS], f32, tag="t1x")
            nc.vector.tensor_scalar_mul(t1[:], Xim[:], 0.1)
            t2 = cs.tile([c.D, c.BS], f32, tag="t2x")
            nc.vector.tensor_scalar_mul(t2[:], Xtx[:], 0.15)
            Xit = cb.tile([c.D, c.BS], f32, tag="Xit")
            nc.vector.tensor_add(Xit[:], t1[:], t2[:])
            nc.vector.tensor_add(Xit[:], Xit[:], sembc[:])

            # ================= C2: session fusion (as v1) =================
            onesf = cb.tile([1, c.D], f32, tag="onesf")
            nc.vector.memset(onesf[:], 1.0)

            def rep_row(row):
                rp = psg.tile([c.D, c.BS], f32, tag="rep", name="rp")
                nc.tensor.matmul(rp[:], lhsT=onesf[:], rhs=row,
                                 start=True, stop=True)
                return rp

            Xim_m = cb.tile([c.D, c.BS], f32, tag="Xim_m")
            Xtx_m = cb.tile([c.D, c.BS], f32, tag="Xtx_m")
            Xit_m = cb.tile([c.D, c.BS], f32, tag="Xit_m")
            ir = rep_row(invd[:])
            nc.vector.tensor_tensor(Xim_m[:], Xim[:], ir[:], op=OP.mult)
            nc.vector.tensor_tensor(Xtx_m[:], Xtx[:], ir[:], op=OP.mult)
            nc.vector.tensor_tensor(Xit_m[:], Xit[:], ir[:], op=OP.mult)

            pgv = psg.tile([c.D, c.BS], f32, tag="pg")
            nc.tensor.matmul(pgv[:], lhsT=wgv[:], rhs=Xim_m[:],
                             start=True, stop=True)
            gv1 = cs.tile([c.D, c.BS], f32, tag="gv1")
            nc.scalar.activation(gv1[:], pgv[:], AF.Sigmoid,
                                 bias=bgv[:, :1], scale=2.0)
            pgt = psg.tile([c.D, c.BS], f32, tag="pg")
            nc.tensor.matmul(pgt[:], lhsT=wgt[:], rhs=Xtx_m[:],
                             start=True, stop=True)
            gt1 = cs.tile([c.D, c.BS], f32, tag="gt1")
            nc.scalar.activation(gt1[:], pgt[:], AF.Sigmoid,
                                 bias=bgt[:, :1], scale=2.0)
            sid = cb.tile([c.D, c.BS], f32, tag="sid")
            std = cb.tile([c.D, c.BS], f32, tag="std")
            nc.vector.tensor_mul(sid[:], Xit_m[:], gv1[:])
            nc.vector.tensor_mul(std[:], Xit_m[:], gt1[:])

            def qc(xin, tag):
                pq = psg.tile([c.D, c.BS], f32, tag="pg")
                nc.tensor.matmul(pq[:], lhsT=wq1[:], rhs=xin[:],
                                 start=True, stop=True)
                th = cs.tile([c.D, c.BS], f32, tag="th")
                nc.scalar.activation(th[:], pq[:], AF.Tanh,
                                     bias=bq1[:, :1], scale=1.0)
                qq = psq.tile([1, c.BS], f32, tag="q0", name="qq" + tag)
                nc.tensor.matmul(qq[:], lhsT=wq2[:], rhs=th[:],
                                 start=True, stop=True)
                qv = cs.tile([1, c.BS], f32, tag="qv" + tag)
                nc.vector.tensor_copy(qv[:], qq[:])
                return qv

            q1v = qc(sid, "a")
            q2v = qc(std, "b")
            qm = cs.tile([1, c.BS], f32, tag="qm")
            nc.vector.tensor_tensor(qm[:], q1v[:], q2v[:], op=OP.max)
            e1 = cs.tile([1, c.BS], f32, tag="e1")
            e2 = cs.tile([1, c.BS], f32, tag="e2")
            nc.vector.tensor_sub(e1[:], q1v[:], qm[:])
            nc.vector.tensor_sub(e2[:], q2v[:], qm[:])
            nc.scalar.activation(e1[:], e1[:], AF.Exp)
            nc.scalar.activation(e2[:], e2[:], AF.Exp)
            esum = cs.tile([1, c.BS], f32, tag="esum")
            nc.vector.tensor_add(esum[:], e1[:], e2[:])
            rsum = cs.tile([1, c.BS], f32, tag="rsum")
            nc.vector.reciprocal(rsum[:], esum[:])
            w1 = cs.tile([1, c.BS], f32, tag="w1")
            w2 = cs.tile([1, c.BS], f32, tag="w2")
            nc.vector.tensor_mul(w1[:], e1[:], rsum[:])
            nc.vector.tensor_mul(w2[:], e2[:], rsum[:])

            com = cb.tile([c.D, c.BS], f32, tag="com")
            tmp1 = cs.tile([c.D, c.BS], f32, tag="tmp1")
            w1r = rep_row(w1[:])
            nc.vector.tensor_tensor(com[:], sid[:], w1r[:], op=OP.mult)
            w2r = rep_row(w2[:])
            nc.vector.tensor_tensor(tmp1[:], std[:], w2r[:], op=OP.mult)
            nc.vector.tensor_add(com[:], com[:], tmp1[:])

            pg2 = psg.tile([c.D, c.BS], f32, tag="pg")
            nc.tensor.matmul(pg2[:], lhsT=wgv[:], rhs=Xit_m[:],
                             start=True, stop=True)
            gv2 = cs.tile([c.D, c.BS], f32, tag="gv2")
            nc.scalar.activation(gv2[:], pg2[:], AF.Sigmoid,
                                 bias=bgv[:, :1], scale=1.0)
            pg3 = psg.tile([c.D, c.BS], f32, tag="pg")
            nc.tensor.matmul(pg3[:], lhsT=wgt[:], rhs=Xit_m[:],
                             start=True, stop=True)
            gt2 = cs.tile([c.D, c.BS], f32, tag="gt2")
            nc.scalar.activation(gt2[:], pg3[:], AF.Sigmoid,
                                 bias=bgt[:, :1], scale=1.0)

            sep = cs.tile([c.D, c.BS], f32, tag="sep")
            nc.vector.tensor_sub(sep[:], sid[:], com[:])
            nc.vector.tensor_mul(sep[:], gv2[:], sep[:])
            sep2 = cs.tile([c.D, c.BS], f32, tag="sep2")
            nc.vector.tensor_sub(sep2[:], std[:], com[:])
            nc.vector.tensor_mul(sep2[:], gt2[:], sep2[:])
            fus = cs.tile([c.D, c.BS], f32, tag="fus")
            nc.vector.tensor_add(fus[:], sep[:], sep2[:])
            nc.vector.tensor_add(fus[:], fus[:], com[:])
            nc.vector.tensor_scalar_mul(fus[:], fus[:], 1.0 / 3.0)
            Xs = cb.tile([c.D, c.BS], f32, tag="Xs")
            nc.vector.tensor_add(Xs[:], Xit_m[:], Xim_m[:])
            nc.vector.tensor_add(Xs[:], Xs[:], Xtx_m[:])
            nc.vector.tensor_add(Xs[:], Xs[:], fus[:])

            # degrees (bf16 G blocks, f32 out)
            idnA = cb.tile([c.L2, c.NPAIR], f32, tag="idnA")
            ideA = cb.tile([c.L2, c.NPAIR], f32, tag="ideA")
            dtmp = cs.tile([c.L2, 2], f32, tag="dtmp")
            for p in range(c.NPAIR):
                nc.vector.reduce_sum(dtmp[:, 0:1],
                                     Gsb[:, p * c.L2:(p + 1) * c.L2], axis=AX.X)
                nc.vector.reduce_sum(dtmp[:, 1:2],
                                     GTsb[:, p * c.L2:(p + 1) * c.L2], axis=AX.X)
                nc.vector.reciprocal(idnA[:, p:p + 1], dtmp[:, 0:1])
                nc.vector.reciprocal(ideA[:, p:p + 1], dtmp[:, 1:2])

            psg_ctx.__exit__(None, None, None)
            psq_ctx.__exit__(None, None, None)

            # ================= C3: hypergraph layers =================
            with (
                tc.tile_pool(name="psR", bufs=2, space="PSUM") as psR,
                tc.tile_pool(name="psE", bufs=2, space="PSUM") as psE,
                tc.tile_pool(name="psT3", bufs=2, space="PSUM") as psT3,
            ):
                for p in range(c.NPAIR):
                    Gp = Gsb[:, p * c.L2:(p + 1) * c.L2]
                    GTp = GTsb[:, p * c.L2:(p + 1) * c.L2]
                    tp_ = psT3.tile([2, c.D], f32, tag="tp")
                    nc.tensor.transpose(tp_[:], Xs[:, 2 * p:2 * p + 2],
                                        identf[:])
                    sp = cs.tile([2, c.D], b16, tag="sp")
                    nc.vector.tensor_copy(sp[:], tp_[:])
                    srep = psR.tile([c.L2, c.D], f32, tag="srep")
                    nc.tensor.matmul(srep[:], lhsT=i2[:], rhs=sp[:],
                                     start=True, stop=True)
                    hcur = h0all[:, p * c.D:(p + 1) * c.D]
                    for lyr in range(2):
                        pe_ = psE.tile([c.L2, c.D], f32, tag="pe")
                        nc.tensor.matmul(pe_[:], lhsT=Gp, rhs=hcur,
                                         start=True, stop=True)
                        ee = cs.tile([c.L2, c.D], b16, tag="ee")
                        nc.scalar.activation(ee[:], pe_[:], AF.Copy,
                                             scale=ideA[:, p:p + 1])
                        ph_ = psE.tile([c.L2, c.D], f32, tag="ph")
                        nc.tensor.matmul(ph_[:], lhsT=GTp, rhs=ee[:],
                                         start=True, stop=True)
                        hs = cs.tile([c.L2, c.D], f32, tag=f"hs{lyr}")
                        nc.scalar.activation(hs[:], ph_[:], AF.Copy,
                                             scale=idnA[:, p:p + 1])
                        if lyr == 0:
                            hh = cs.tile([c.L2, c.D], b16, tag="hh0")
                            nc.vector.tensor_add(hh[:], hs[:], srep[:])
                            hcur = hh[:]
                        else:
                            hh2 = cs.tile([c.L2, c.D], f32, tag="hh1")
                            nc.vector.tensor_add(hh2[:], hs[:], srep[:])
                            nc.sync.dma_start(
                                outH[2 * p:2 * p + 2].rearrange(
                                    "b l d -> (b l) d"), hh2[:])
    nc.compile()
    return nc


_CACHE = {}


def _get_program(c: Cfg):
    key = (c.N, c.B)
    if key not in _CACHE:
        _CACHE[key] = build_program(c)
    return _CACHE[key]


def _prep_inputs(c: Cfg, inputs, item, mask_item, Hs, emb_table, img_table,
                 txt_table, img_W, img_b, txt_W, txt_b, gate_v_W, gate_v_b,
                 gate_t_W, gate_t_b, qc_W1, qc_b1, qc_W2):
    f32 = np.float32
    inputs = np.asarray(inputs); item = np.asarray(item)
    # U = referenced vocab ids (>=1 in padded space; id 0 is the pad row)
    U = np.unique(np.concatenate([inputs.ravel(), item.ravel()]))
    U = U[U > 0].astype(np.int64)
    NU = len(U)
    assert NU <= c.NU, f"unique referenced rows {NU} > capacity {c.NU}"
    Upad = np.concatenate([U, np.full(c.NU - NU, U[-1], np.int64)])
    pos = np.zeros(c.N + 1, np.int32)
    pos[U] = np.arange(1, NU + 1, dtype=np.int32)
    h0x = pos[inputs]                       # [B, L] U-space indices
    ssx = pos[item]

    rows = Upad - 1
    rawcat = np.zeros((c.NF, c.RAWW), dtype=BF16)
    rawcat[1:, :c.IMG] = img_table[rows].astype(BF16)
    rawcat[1:, c.IMG:c.IMG + c.TXT] = txt_table[rows].astype(BF16)
    rawcat[1:, c.IMG + c.TXT:] = emb_table[rows].astype(BF16)

    maskf = np.asarray(mask_item).astype(f32)
    bcomb = (0.1 * img_b + 0.15 * txt_b).astype(f32).reshape(c.D, 1)
    # bias baked into the transposed emb rows (applies to every real row)
    in_maps = []
    for kk in range(c.NC):
        Uc = Upad[kk * c.UC:(kk + 1) * c.UC] - 1
        b0, b1 = kk * c.BS, (kk + 1) * c.BS
        Hk = np.asarray(Hs[b0:b1]).astype(f32)
        mk = maskf[b0:b1]
        Gbd = np.zeros((c.NPAIR, c.L2, c.L2), f32)
        GTbd = np.zeros((c.NPAIR, c.L2, c.L2), f32)
        Mbd = np.zeros((c.NPAIR, c.L2, 2), f32)
        for p in range(c.NPAIR):
            Gbd[p, :c.L, :c.L] = Hk[2 * p]
            Gbd[p, c.L:, c.L:] = Hk[2 * p + 1]
            GTbd[p, :c.L, :c.L] = Hk[2 * p].T
            GTbd[p, c.L:, c.L:] = Hk[2 * p + 1].T
            Mbd[p, :c.L, 0] = mk[2 * p]
            Mbd[p, c.L:, 1] = mk[2 * p + 1]
        ind2 = np.zeros((2, c.L2), f32)
        ind2[0, :c.L] = 1.0
        ind2[1, c.L:] = 1.0
        mind = mk * (item[b0:b1] > 0)
        in_maps.append({
            "imgRT": np.ascontiguousarray(img_table[Uc].T).astype(BF16),
            "txtRT": np.ascontiguousarray(txt_table[Uc].T).astype(BF16),
            "embRT": (np.ascontiguousarray(emb_table[Uc].T) + bcomb).astype(f32),
            "wiN": img_W.astype(BF16), "wtN": txt_W.astype(BF16),
            "wi01": (0.1 * img_W).astype(BF16),
            "wt015": (0.15 * txt_W).astype(BF16),
            "imgbR": img_b.reshape(1, c.D).astype(BF16),
            "txtbR": txt_b.reshape(1, c.D).astype(BF16),
            "rawcat": rawcat,
            "gvW": gate_v_W.astype(f32), "gvB": gate_v_b.reshape(c.D, 1).astype(f32),
            "gtW": gate_t_W.astype(f32), "gtB": gate_t_b.reshape(c.D, 1).astype(f32),
            "q1W": qc_W1.astype(f32), "q1B": qc_b1.reshape(c.D, 1).astype(f32),
            "q2W": qc_W2.astype(f32),
            "Gbd": Gbd.astype(BF16), "GTbd": GTbd.astype(BF16),
            "Mbd16": Mbd.astype(BF16),
            "mkT": np.ascontiguousarray(mk.T).astype(BF16),
            "mindT": np.ascontiguousarray(mind.T).astype(BF16),
            "ind2": ind2.astype(BF16),
            "h0idx": h0x[b0:b1].reshape(c.NPAIR, c.L2, 1),
            "ssidx": ssx[b0:b1].reshape(c.NPAIR, c.L2, 1),
        })
    return in_maps


def run(c: Cfg, trace=False, **inputs):
    nc = _get_program(c)
    in_maps = _prep_inputs(c, **{k: np.asarray(v) for k, v in inputs.items()})
    res = bass_utils.run_bass_kernel_spmd(
        nc, in_maps, core_ids=list(range(c.NC)), trace=trace)
    out = np.concatenate([r["outH"] for r in res.results], axis=0)
    return out.astype(np.float32), res


def kernel(**inputs):
    out, _ = run(REAL, trace=False, **inputs)
    return out


# revision 17
# speedup vs baseline: 1.0739x; 1.0739x over previous
"""Trainium2 Bass kernel for the gnn_message_passing problem.

Strategy (8 NeuronCores, SPMD), v2:
  - Host computes U = unique(inputs|item) (~32k of 50k vocab rows actually
    referenced).  Each core projects a 4096-row slice of U to the 128-dim
    item embedding (emb + 0.1*(img@Wi+bi) + 0.15*(txt@Wt+bt)) using
    weights-stationary N=512 bf16 matmuls in transposed orientation
    ([dout, rows]), then PE-transposes back to row-major bf16.
  - ONE bf16 AllGather of the compact item table (8.4MB vs 51MB in v1).
    The Tvis (img|txt projected) all-gather is eliminated entirely via
    linearity: session_img = (W^T @ masked_row_sum + b*cnt)/denom, so the
    session means are computed from RAW table rows gathered per batch
    shard from a U-compacted [32769, 1896] bf16 concat table, with
    mask-stationary [100,2]x[100,512] matmuls (few, large instructions).
  - Batch-sharded phase C: session fusion math in transposed [128, 64]
    layout (as v1), hypergraph layers per session pair in bf16.
"""

import sys

sys.path.insert(0, "/opt/trn_rl_repo")

import numpy as np
import ml_dtypes

import concourse.bass as bass
import concourse.bacc as bacc
import concourse.mybir as mybir
import concourse.tile as tile
from concourse import bass_utils

BF16 = ml_dtypes.bfloat16


class Cfg:
    def __init__(self):
        self.N = 50000
        self.D = 128
        self.IMG = 1000
        self.TXT = 768
        self.B = 512
        self.L = 50
        self.NC = 8
        self.UC = 4096                 # U rows projected per core
        self.NU = self.NC * self.UC    # 32768 capacity
        self.NF = 1 + self.NU          # padded table rows (row 0 = zeros)
        self.BS = self.B // self.NC    # 64 sessions per core
        self.NPAIR = self.BS // 2      # 32
        self.L2 = 2 * self.L           # 100
        self.KI = 8                    # img k-chunks of 125
        self.KIW = 125
        self.KT = 6                    # txt k-chunks of 128
        self.KTW = 128
        self.RAWW = self.IMG + self.TXT + self.D   # 1896
        self.ACH = self.UC // 512      # 8 phase-A chunks of 512 rows


REAL = Cfg()


def build_program(c: Cfg):
    f32 = mybir.dt.float32
    b16 = mybir.dt.bfloat16
    i32 = mybir.dt.int32
    AF = mybir.ActivationFunctionType
    AX = mybir.AxisListType
    OP = mybir.AluOpType

    nc = bacc.Bacc("TRN2", target_bir_lowering=False, debug=False,
                   num_devices=c.NC)

    def ein(nm, sh, dt):
        return nc.dram_tensor(nm, sh, dt, kind="ExternalInput")

    imgRT = ein("imgRT", [c.KIW, c.ACH, c.KI, 512], b16)  # [f, cc, k, v]
    txtRT = ein("txtRT", [c.KTW, c.ACH, c.KT, 512], b16)
    embRT = ein("embRT", [c.D, c.ACH, 512], f32)          # bias baked in
    wiN = ein("wiN", [c.IMG, c.D], b16)         # img_W
    wtN = ein("wtN", [c.TXT, c.D], b16)
    wi01 = ein("wi01", [c.IMG, c.D], b16)       # 0.1 * img_W
    wt015 = ein("wt015", [c.TXT, c.D], b16)     # 0.15 * txt_W
    imgbR = ein("imgbR", [1, c.D], b16)
    txtbR = ein("txtbR", [1, c.D], b16)
    rawcat = ein("rawcat", [c.NF, c.RAWW], b16)  # U-compacted img|txt|emb
    gvW = ein("gvW", [c.D, c.D], f32)
    gvB = ein("gvB", [c.D, 1], f32)
    gtW = ein("gtW", [c.D, c.D], f32)
    gtB = ein("gtB", [c.D, 1], f32)
    q1W = ein("q1W", [c.D, c.D], f32)
    q1B = ein("q1B", [c.D, 1], f32)
    q2W = ein("q2W", [c.D, 1], f32)
    Gbd = ein("Gbd", [c.NPAIR, c.L2, c.L2], b16)
    GTbd = ein("GTbd", [c.NPAIR, c.L2, c.L2], b16)
    Mbd16 = ein("Mbd16", [c.NPAIR, c.L2, 2], b16)
    mkT = ein("mkT", [c.L, c.BS], b16)          # mask.T
    mindT = ein("mindT", [c.L, c.BS], b16)      # (mask * (item>0)).T
    ind2 = ein("ind2", [2, c.L2], b16)
    h0idx = ein("h0idx", [c.NPAIR, c.L2, 1], i32)
    ssidx = ein("ssidx", [c.NPAIR, c.L2, 1], i32)

    outH = nc.dram_tensor("outH", [c.BS, c.L, c.D], f32, kind="ExternalOutput")

    localI = nc.dram_tensor("localI", [c.UC, c.D], b16)
    XsDram = nc.dram_tensor("XsDram", [c.BS, c.D], b16)
    Titem = nc.dram_tensor("Titem", [c.NF, c.D], b16, addr_space="Shared")

    rg = [list(range(c.NC))]
    # raw-concat column chunks for transposes / projection
    # img: 8 x 125, txt: 6 x 128, emb: 1 x 128
    CH = [(k * c.KIW, c.KIW) for k in range(c.KI)]
    CH += [(c.IMG + k * c.KTW, c.KTW) for k in range(c.KT)]
    CH += [(c.IMG + c.TXT, c.D)]
    # session-sum segments (psum free-dim <= 512)
    SEG = [(0, 512), (512, 512), (1024, 512), (1536, 360)]

    with tile.TileContext(nc) as tc:
        with (
            tc.tile_pool(name="wpool", bufs=1) as wp,
            tc.tile_pool(name="apool", bufs=2) as ap,
            tc.tile_pool(name="ostg", bufs=2) as ost,
            tc.tile_pool(name="cbig", bufs=1) as cb,
            tc.tile_pool(name="cgat", bufs=4) as cg,
            tc.tile_pool(name="csml", bufs=3) as cs,
        ):
            # ---- weights / constants ----
            wi01t = [wp.tile([c.KIW, c.D], b16, tag=f"wi01_{k}", name=f"wi01_{k}")
                     for k in range(c.KI)]
            wt015t = [wp.tile([c.KTW, c.D], b16, tag=f"wt015_{k}", name=f"wt015_{k}")
                      for k in range(c.KT)]
            wiNt = [wp.tile([c.KIW, c.D], b16, tag=f"wiN_{k}", name=f"wiN_{k}")
                    for k in range(c.KI)]
            wtNt = [wp.tile([c.KTW, c.D], b16, tag=f"wtN_{k}", name=f"wtN_{k}")
                    for k in range(c.KT)]
            for k in range(c.KI):
                nc.sync.dma_start(wi01t[k][:], wi01[k * c.KIW:(k + 1) * c.KIW, :])
                nc.sync.dma_start(wiNt[k][:], wiN[k * c.KIW:(k + 1) * c.KIW, :])
            for k in range(c.KT):
                nc.sync.dma_start(wt015t[k][:], wt015[k * c.KTW:(k + 1) * c.KTW, :])
                nc.sync.dma_start(wtNt[k][:], wtN[k * c.KTW:(k + 1) * c.KTW, :])
            bir_ = wp.tile([1, c.D], b16, tag="bir")
            btr = wp.tile([1, c.D], b16, tag="btr")
            nc.sync.dma_start(bir_[:], imgbR[:])
            nc.sync.dma_start(btr[:], txtbR[:])
            ident16 = wp.tile([128, 128], b16, tag="id16")
            identf = wp.tile([128, 128], f32, tag="idf")
            from concourse.masks import make_identity
            make_identity(nc, ident16[:])
            make_identity(nc, identf[:])

            # zero row 0 of Titem
            zi = wp.tile([1, c.D], b16, tag="zi")
            nc.vector.memset(zi[:], 0.0)
            nc.sync.dma_start(Titem[0:1, :], zi[:])

            # phase-C persistent loads (start DMAs early; no dep on phase A)
            m16 = cb.tile([c.L2, c.NPAIR * 2], b16, tag="m16")
            nc.sync.dma_start(
                m16[:].rearrange("l (p j) -> l p j", p=c.NPAIR),
                Mbd16.rearrange("p l j -> l p j"))
            hix = cb.tile([c.L2, c.NPAIR], i32, tag="hix")
            six = cb.tile([c.L2, c.NPAIR], i32, tag="six")
            nc.sync.dma_start(hix[:], h0idx.rearrange("p l o -> l (p o)"))
            nc.sync.dma_start(six[:], ssidx.rearrange("p l o -> l (p o)"))
            mkTt = cb.tile([c.L, c.BS], b16, tag="mkT")
            minTt = cb.tile([c.L, c.BS], b16, tag="minT")
            nc.sync.dma_start(mkTt[:], mkT[:])
            nc.sync.dma_start(minTt[:], mindT[:])
            i2 = cb.tile([2, c.L2], b16, tag="i2")
            nc.sync.dma_start(i2[:], ind2[:])
            ones50 = cb.tile([c.L, 1], b16, tag="ones50")
            nc.vector.memset(ones50[:], 1.0)
            wgv = cb.tile([c.D, c.D], f32, tag="wgv")
            wgt = cb.tile([c.D, c.D], f32, tag="wgt")
            wq1 = cb.tile([c.D, c.D], f32, tag="wq1")
            wq2 = cb.tile([c.D, 1], f32, tag="wq2")
            bgv = cb.tile([c.D, 1], f32, tag="bgv")
            bgt = cb.tile([c.D, 1], f32, tag="bgt")
            bq1 = cb.tile([c.D, 1], f32, tag="bq1")
            nc.sync.dma_start(wgv[:], gvW[:])
            nc.sync.dma_start(wgt[:], gtW[:])
            nc.sync.dma_start(wq1[:], q1W[:])
            nc.sync.dma_start(wq2[:], q2W[:])
            nc.sync.dma_start(bgv[:], gvB[:])
            nc.sync.dma_start(bgt[:], gtB[:])
            nc.sync.dma_start(bq1[:], q1B[:])

            psq_ctx = tc.tile_pool(name="psq", bufs=1, space="PSUM")
            psq = psq_ctx.__enter__()
            # denom / cnt row vectors
            dT = psq.tile([1, c.BS], f32, tag="q0", name="dT")
            nc.tensor.matmul(dT[:], lhsT=ones50[:], rhs=mkTt[:],
                             start=True, stop=True)
            invd = cb.tile([1, c.BS], f32, tag="invd")
            nc.vector.reciprocal(invd[:], dT[:])
            cT = psq.tile([1, c.BS], f32, tag="q0", name="cT")
            nc.tensor.matmul(cT[:], lhsT=ones50[:], rhs=minTt[:],
                             start=True, stop=True)
            cntR = cb.tile([1, c.BS], b16, tag="cntR")
            nc.vector.tensor_copy(cntR[:], cT[:])

            # ================= Phase A: project U_c rows =================
            psA_ctx = tc.tile_pool(name="psA", bufs=2, space="PSUM")
            psA = psA_ctx.__enter__()
            psB_ctx = tc.tile_pool(name="psB", bufs=2, space="PSUM")
            psB = psB_ctx.__enter__()
            for cc in range(c.ACH):
                v0 = cc * 512
                ai = ap.tile([c.KIW, c.KI * 512], b16, tag="ai")
                nc.sync.dma_start(
                    ai[:].rearrange("f (k v) -> f k v", k=c.KI),
                    imgRT[:, cc, :, :])
                at = ap.tile([c.KTW, c.KT * 512], b16, tag="at")
                nc.sync.dma_start(
                    at[:].rearrange("f (k v) -> f k v", k=c.KT),
                    txtRT[:, cc, :, :])
                ae = ap.tile([c.D, 512], f32, tag="ae")
                nc.sync.dma_start(ae[:], embRT[:, cc, :])
                pst = psA.tile([c.D, 512], f32, tag="psA")
                for k in range(c.KI):
                    nc.tensor.matmul(pst[:], lhsT=wi01t[k][:],
                                     rhs=ai[:, k * 512:(k + 1) * 512],
                                     start=(k == 0), stop=False)
                for k in range(c.KT):
                    nc.tensor.matmul(pst[:], lhsT=wt015t[k][:],
                                     rhs=at[:, k * 512:(k + 1) * 512],
                                     start=False, stop=(k == c.KT - 1))
                io = ost.tile([c.D, 512], f32, tag="io")
                nc.vector.tensor_add(io[:], pst[:], ae[:])
                st = ost.tile([128, 512], b16, tag="st")
                for j in range(4):
                    trj = psB.tile([128, 128], f32, tag="trj")
                    nc.tensor.transpose(trj[:], io[:, j * 128:(j + 1) * 128],
                                        identf[:])
                    if j % 2 == 0:
                        nc.scalar.copy(st[:, j * 128:(j + 1) * 128], trj[:])
                    else:
                        nc.vector.tensor_copy(st[:, j * 128:(j + 1) * 128], trj[:])
                nc.sync.dma_start(
                    localI[v0:v0 + 512, :].rearrange("(n p) d -> p n d", n=4),
                    st[:].rearrange("p (n d) -> p n d", n=4))

            psB_ctx.__exit__(None, None, None)
            psA_ctx.__exit__(None, None, None)

            # ================= AllGather (bf16 item table) =================
            nc.gpsimd.collective_compute(
                "AllGather", mybir.AluOpType.bypass, replica_groups=rg,
                ins=[localI[:].opt()], outs=[Titem[1:c.NF, :].opt()])

            # h0 gathers (depend on AG; DMA overlaps session sums below)
            h0all = cb.tile([c.L2, c.NPAIR * c.D], b16, tag="h0all")
            for p in range(c.NPAIR):
                nc.gpsimd.indirect_dma_start(
                    out=h0all[:, p * c.D:(p + 1) * c.D], out_offset=None,
                    in_=Titem[:],
                    in_offset=bass.IndirectOffsetOnAxis(
                        ap=hix[:, p:p + 1], axis=0))

            # ========== session raw sums (independent of phase A / AG) =====
            Sraw = cb.tile([c.BS, c.RAWW], f32, tag="Sraw")
            psS_ctx = tc.tile_pool(name="psS", bufs=2, space="PSUM")
            psS = psS_ctx.__enter__()
            for p in range(c.NPAIR):
                gvi = cg.tile([c.L2, c.RAWW], b16, tag="gvi")
                nc.gpsimd.indirect_dma_start(
                    out=gvi[:], out_offset=None, in_=rawcat[:],
                    in_offset=bass.IndirectOffsetOnAxis(
                        ap=six[:, p:p + 1], axis=0))
                stg = cs.tile([2, c.RAWW], f32, tag="stg")
                for jj, (s0, w) in enumerate(SEG):
                    pss = psS.tile([2, 512], f32, tag="pss")
                    nc.tensor.matmul(pss[:, 0:w], lhsT=m16[:, 2 * p:2 * p + 2],
                                     rhs=gvi[:, s0:s0 + w],
                                     start=True, stop=True)
                    if jj % 2 == 0:
                        nc.scalar.copy(stg[:, s0:s0 + w], pss[:, 0:w])
                    else:
                        nc.vector.tensor_copy(stg[:, s0:s0 + w], pss[:, 0:w])
                nc.sync.dma_start(Sraw[2 * p:2 * p + 2, :], stg[:])

            Gsb = cb.tile([c.L2, c.NPAIR * c.L2], b16, tag="Gsb")
            GTsb = cb.tile([c.L2, c.NPAIR * c.L2], b16, tag="GTsb")
            nc.sync.dma_start(
                Gsb[:].rearrange("l (p e) -> l p e", p=c.NPAIR),
                Gbd.rearrange("p l e -> l p e"))
            nc.sync.dma_start(
                GTsb[:].rearrange("l (p e) -> l p e", p=c.NPAIR),
                GTbd.rearrange("p l e -> l p e"))

            # transpose Sraw -> SrawT chunks [d-chunk, 64] bf16
            psT_ctx = tc.tile_pool(name="psT", bufs=2, space="PSUM")
            psT = psT_ctx.__enter__()
            SrawT = cb.tile([128, len(CH) * c.BS], b16, tag="SrawT")
            for ch, (s0, w) in enumerate(CH):
                trS = psT.tile([128, c.BS], f32, tag="trS")
                nc.tensor.transpose(trS[0:w, :], Sraw[:, s0:s0 + w],
                                    identf[0:c.BS, 0:c.BS])
                nc.scalar.copy(SrawT[0:w, ch * c.BS:(ch + 1) * c.BS],
                               trS[0:w, :])
            psT_ctx.__exit__(None, None, None)
            psS_ctx.__exit__(None, None, None)

            # ========== project session sums:  X = W^T S + b*cnt ==========
            psg_ctx = tc.tile_pool(name="psg", bufs=2, space="PSUM")
            psg = psg_ctx.__enter__()
            XimP = psg.tile([c.D, c.BS], f32, tag="px", name="XimP")
            for k in range(c.KI):
                nc.tensor.matmul(XimP[:], lhsT=wiNt[k][:],
                                 rhs=SrawT[0:c.KIW, k * c.BS:(k + 1) * c.BS],
                                 start=(k == 0), stop=False)
            nc.tensor.matmul(XimP[:], lhsT=bir_[:], rhs=cntR[:],
                             start=False, stop=True)
            Xim = cb.tile([c.D, c.BS], f32, tag="Xim")
            nc.scalar.copy(Xim[:], XimP[:])
            XtxP = psg.tile([c.D, c.BS], f32, tag="px", name="XtxP")
            for k in range(c.KT):
                nc.tensor.matmul(
                    XtxP[:], lhsT=wtNt[k][:],
                    rhs=SrawT[0:c.KTW, (c.KI + k) * c.BS:(c.KI + k + 1) * c.BS],
                    start=(k == 0), stop=False)
            nc.tensor.matmul(XtxP[:], lhsT=btr[:], rhs=cntR[:],
                             start=False, stop=True)
            Xtx = cb.tile([c.D, c.BS], f32, tag="Xtx")
            nc.scalar.copy(Xtx[:], XtxP[:])
            # Xit = Semb + 0.1 Xim + 0.15 Xtx
            sembc = cs.tile([c.D, c.BS], f32, tag="sembc")
            nc.scalar.copy(sembc[:],
                           SrawT[:, (c.KI + c.KT) * c.BS:(c.KI + c.KT + 1) * c.BS])
            t1 = cs.tile([c.D, c.BS], f32, tag="t1x")
            nc.vector.tensor_scalar_mul(t1[:], Xim[:], 0.1)
            t2 = cs.tile([c.D, c.BS], f32, tag="t2x")
            nc.vector.tensor_scalar_mul(t2[:], Xtx[:], 0.15)
            Xit = cb.tile([c.D, c.BS], f32, tag="Xit")
            nc.vector.tensor_add(Xit[:], t1[:], t2[:])
            nc.vector.tensor_add(Xit[:], Xit[:], sembc[:])

            # ================= C2: session fusion (as v1) =================
            onesf = cb.tile([1, c.D], f32, tag="onesf")
            nc.vector.memset(onesf[:], 1.0)

            def rep_row(row):
                rp = psg.tile([c.D, c.BS], f32, tag="rep", name="rp")
                nc.tensor.matmul(rp[:], lhsT=onesf[:], rhs=row,
                                 start=True, stop=True)
                return rp

            Xim_m = cb.tile([c.D, c.BS], f32, tag="Xim_m")
            Xtx_m = cb.tile([c.D, c.BS], f32, tag="Xtx_m")
            Xit_m = cb.tile([c.D, c.BS], f32, tag="Xit_m")
            ir = rep_row(invd[:])
            nc.vector.tensor_tensor(Xim_m[:], Xim[:], ir[:], op=OP.mult)
            nc.vector.tensor_tensor(Xtx_m[:], Xtx[:], ir[:], op=OP.mult)
            nc.vector.tensor_tensor(Xit_m[:], Xit[:], ir[:], op=OP.mult)

            pgv = psg.tile([c.D, c.BS], f32, tag="pg")
            nc.tensor.matmul(pgv[:], lhsT=wgv[:], rhs=Xim_m[:],
                             start=True, stop=True)
            gv1 = cs.tile([c.D, c.BS], f32, tag="gv1")
            nc.scalar.activation(gv1[:], pgv[:], AF.Sigmoid,
                                 bias=bgv[:, :1], scale=2.0)
            pgt = psg.tile([c.D, c.BS], f32, tag="pg")
            nc.tensor.matmul(pgt[:], lhsT=wgt[:], rhs=Xtx_m[:],
                             start=True, stop=True)
            gt1 = cs.tile([c.D, c.BS], f32, tag="gt1")
            nc.scalar.activation(gt1[:], pgt[:], AF.Sigmoid,
                                 bias=bgt[:, :1], scale=2.0)
            sid = cb.tile([c.D, c.BS], f32, tag="sid")
            std = cb.tile([c.D, c.BS], f32, tag="std")
            nc.vector.tensor_mul(sid[:], Xit_m[:], gv1[:])
            nc.vector.tensor_mul(std[:], Xit_m[:], gt1[:])

            def qc(xin, tag):
                pq = psg.tile([c.D, c.BS], f32, tag="pg")
                nc.tensor.matmul(pq[:], lhsT=wq1[:], rhs=xin[:],
                                 start=True, stop=True)
                th = cs.tile([c.D, c.BS], f32, tag="th")
                nc.scalar.activation(th[:], pq[:], AF.Tanh,
                                     bias=bq1[:, :1], scale=1.0)
                qq = psq.tile([1, c.BS], f32, tag="q0", name="qq" + tag)
                nc.tensor.matmul(qq[:], lhsT=wq2[:], rhs=th[:],
                                 start=True, stop=True)
                qv = cs.tile([1, c.BS], f32, tag="qv" + tag)
                nc.vector.tensor_copy(qv[:], qq[:])
                return qv

            q1v = qc(sid, "a")
            q2v = qc(std, "b")
            qm = cs.tile([1, c.BS], f32, tag="qm")
            nc.vector.tensor_tensor(qm[:], q1v[:], q2v[:], op=OP.max)
            e1 = cs.tile([1, c.BS], f32, tag="e1")
            e2 = cs.tile([1, c.BS], f32, tag="e2")
            nc.vector.tensor_sub(e1[:], q1v[:], qm[:])
            nc.vector.tensor_sub(e2[:], q2v[:], qm[:])
            nc.scalar.activation(e1[:], e1[:], AF.Exp)
            nc.scalar.activation(e2[:], e2[:], AF.Exp)
            esum = cs.tile([1, c.BS], f32, tag="esum")
            nc.vector.tensor_add(esum[:], e1[:], e2[:])
            rsum = cs.tile([1, c.BS], f32, tag="rsum")
            nc.vector.reciprocal(rsum[:], esum[:])
            w1 = cs.tile([1, c.BS], f32, tag="w1")
            w2 = cs.tile([1, c.BS], f32, tag="w2")
            nc.vector.tensor_mul(w1[:], e1[:], rsum[:])
            nc.vector.tensor_mul(w2[:], e2[:], rsum[:])

            com = cb.tile([c.D, c.BS], f32, tag="com")
            tmp1 = cs.tile([c.D, c.BS], f32, tag="tmp1")
            w1r = rep_row(w1[:])
            nc.vector.tensor_tensor(com[:], sid[:], w1r[:], op=OP.mult)
            w2r = rep_row(w2[:])
            nc.vector.tensor_tensor(tmp1[:], std[:], w2r[:], op=OP.mult)
            nc.vector.tensor_add(com[:], com[:], tmp1[:])

            pg2 = psg.tile([c.D, c.BS], f32, tag="pg")
            nc.tensor.matmul(pg2[:], lhsT=wgv[:], rhs=Xit_m[:],
                             start=True, stop=True)
            gv2 = cs.tile([c.D, c.BS], f32, tag="gv2")
            nc.scalar.activation(gv2[:], pg2[:], AF.Sigmoid,
                                 bias=bgv[:, :1], scale=1.0)
            pg3 = psg.tile([c.D, c.BS], f32, tag="pg")
            nc.tensor.matmul(pg3[:], lhsT=wgt[:], rhs=Xit_m[:],
                             start=True, stop=True)
            gt2 = cs.tile([c.D, c.BS], f32, tag="gt2")
            nc.scalar.activation(gt2[:], pg3[:], AF.Sigmoid,
                                 bias=bgt[:, :1], scale=1.0)

            sep = cs.tile([c.D, c.BS], f32, tag="sep")
            nc.vector.tensor_sub(sep[:], sid[:], com[:])
            nc.vector.tensor_mul(sep[:], gv2[:], sep[:])
            sep2 = cs.tile([c.D, c.BS], f32, tag="sep2")
            nc.vector.tensor_sub(sep2[:], std[:], com[:])
            nc.vector.tensor_mul(sep2[:], gt2[:], sep2[:])
            fus = cs.tile([c.D, c.BS], f32, tag="fus")
            nc.vector.tensor_add(fus[:], sep[:], sep2[:])
            nc.vector.tensor_add(fus[:], fus[:], com[:])
            nc.vector.tensor_scalar_mul(fus[:], fus[:], 1.0 / 3.0)
            Xs = cb.tile([c.D, c.BS], f32, tag="Xs")
            nc.vector.tensor_add(Xs[:], Xit_m[:], Xim_m[:])
            nc.vector.tensor_add(Xs[:], Xs[:], Xtx_m[:])
            nc.vector.tensor_add(Xs[:], Xs[:], fus[:])

            # degrees (bf16 G blocks, f32 out)
            idnA = cb.tile([c.L2, c.NPAIR], f32, tag="idnA")
            ideA = cb.tile([c.L2, c.NPAIR], f32, tag="ideA")
            dtmp = cs.tile([c.L2, 2], f32, tag="dtmp")
            for p in range(c.NPAIR):
                nc.vector.reduce_sum(dtmp[:, 0:1],
                                     Gsb[:, p * c.L2:(p + 1) * c.L2], axis=AX.X)
                nc.vector.reduce_sum(dtmp[:, 1:2],
                                     GTsb[:, p * c.L2:(p + 1) * c.L2], axis=AX.X)
                nc.vector.reciprocal(idnA[:, p:p + 1], dtmp[:, 0:1])
                nc.vector.reciprocal(ideA[:, p:p + 1], dtmp[:, 1:2])

            # Xs -> per-pair session rows [2, 32*128] via one transpose + DMA
            trx = psg.tile([c.BS, c.D], f32, tag="px", name="trx")
            nc.tensor.transpose(trx[:], Xs[:], identf[:])
            XsSb = cs.tile([c.BS, c.D], b16, tag="XsSb")
            nc.vector.tensor_copy(XsSb[:], trx[:])
            nc.sync.dma_start(XsDram[:], XsSb[:])
            Xrows = cb.tile([2, c.NPAIR * c.D], b16, tag="Xrows")
            nc.sync.dma_start(
                Xrows[:].rearrange("j (p d) -> j p d", p=c.NPAIR),
                XsDram.rearrange("(p j) d -> j p d", p=c.NPAIR))

            psg_ctx.__exit__(None, None, None)
            psq_ctx.__exit__(None, None, None)

            # ================= C3: hypergraph layers =================
            with (
                tc.tile_pool(name="psR", bufs=2, space="PSUM") as psR,
                tc.tile_pool(name="psE", bufs=2, space="PSUM") as psE,
            ):
                for p in range(c.NPAIR):
                    Gp = Gsb[:, p * c.L2:(p + 1) * c.L2]
                    GTp = GTsb[:, p * c.L2:(p + 1) * c.L2]
                    srep = psR.tile([c.L2, c.D], f32, tag="srep")
                    nc.tensor.matmul(srep[:], lhsT=i2[:],
                                     rhs=Xrows[:, p * c.D:(p + 1) * c.D],
                                     start=True, stop=True)
                    hcur = h0all[:, p * c.D:(p + 1) * c.D]
                    for lyr in range(2):
                        pe_ = psE.tile([c.L2, c.D], f32, tag="pe")
                        nc.tensor.matmul(pe_[:], lhsT=Gp, rhs=hcur,
                                         start=True, stop=True)
                        ee = cs.tile([c.L2, c.D], b16, tag="ee")
                        nc.scalar.activation(ee[:], pe_[:], AF.Copy,
                                             scale=ideA[:, p:p + 1])
                        ph_ = psE.tile([c.L2, c.D], f32, tag="ph")
                        nc.tensor.matmul(ph_[:], lhsT=GTp, rhs=ee[:],
                                         start=True, stop=True)
                        hs = cs.tile([c.L2, c.D], f32, tag=f"hs{lyr}")
                        nc.scalar.activation(hs[:], ph_[:], AF.Copy,
                                             scale=idnA[:, p:p + 1])
                        if lyr == 0:
                            hh = cs.tile([c.L2, c.D], b16, tag="hh0")
                            nc.vector.tensor_add(hh[:], hs[:], srep[:])
                            hcur = hh[:]
                        else:
                            hh2 = cs.tile([c.L2, c.D], f32, tag="hh1")
                            nc.vector.tensor_add(hh2[:], hs[:], srep[:])
                            nc.sync.dma_start(
                                outH[2 * p:2 * p + 2].rearrange(
                                    "b l d -> (b l) d"), hh2[:])
    nc.compile()
    return nc


_CACHE = {}


def _get_program(c: Cfg):
    key = (c.N, c.B)
    if key not in _CACHE:
        _CACHE[key] = build_program(c)
    return _CACHE[key]


def _prep_inputs(c: Cfg, inputs, item, mask_item, Hs, emb_table, img_table,
                 txt_table, img_W, img_b, txt_W, txt_b, gate_v_W, gate_v_b,
                 gate_t_W, gate_t_b, qc_W1, qc_b1, qc_W2):
    f32 = np.float32
    inputs = np.asarray(inputs); item = np.asarray(item)
    # U = referenced vocab ids (>=1 in padded space; id 0 is the pad row)
    U = np.unique(np.concatenate([inputs.ravel(), item.ravel()]))
    U = U[U > 0].astype(np.int64)
    NU = len(U)
    assert NU <= c.NU, f"unique referenced rows {NU} > capacity {c.NU}"
    Upad = np.concatenate([U, np.full(c.NU - NU, U[-1], np.int64)])
    pos = np.zeros(c.N + 1, np.int32)
    pos[U] = np.arange(1, NU + 1, dtype=np.int32)
    h0x = pos[inputs]                       # [B, L] U-space indices
    ssx = pos[item]

    rows = Upad - 1
    rawcat = np.zeros((c.NF, c.RAWW), dtype=BF16)
    rawcat[1:, :c.IMG] = img_table[rows].astype(BF16)
    rawcat[1:, c.IMG:c.IMG + c.TXT] = txt_table[rows].astype(BF16)
    rawcat[1:, c.IMG + c.TXT:] = emb_table[rows].astype(BF16)

    maskf = np.asarray(mask_item).astype(f32)
    bcomb = (0.1 * img_b + 0.15 * txt_b).astype(f32).reshape(c.D, 1)
    # bias baked into the transposed emb rows (applies to every real row)
    in_maps = []
    for kk in range(c.NC):
        Uc = Upad[kk * c.UC:(kk + 1) * c.UC] - 1
        b0, b1 = kk * c.BS, (kk + 1) * c.BS
        Hk = np.asarray(Hs[b0:b1]).astype(f32)
        mk = maskf[b0:b1]
        Gbd = np.zeros((c.NPAIR, c.L2, c.L2), f32)
        GTbd = np.zeros((c.NPAIR, c.L2, c.L2), f32)
        Mbd = np.zeros((c.NPAIR, c.L2, 2), f32)
        for p in range(c.NPAIR):
            Gbd[p, :c.L, :c.L] = Hk[2 * p]
            Gbd[p, c.L:, c.L:] = Hk[2 * p + 1]
            GTbd[p, :c.L, :c.L] = Hk[2 * p].T
            GTbd[p, c.L:, c.L:] = Hk[2 * p + 1].T
            Mbd[p, :c.L, 0] = mk[2 * p]
            Mbd[p, c.L:, 1] = mk[2 * p + 1]
        ind2 = np.zeros((2, c.L2), f32)
        ind2[0, :c.L] = 1.0
        ind2[1, c.L:] = 1.0
        mind = mk * (item[b0:b1] > 0)
        in_maps.append({
            "imgRT": np.ascontiguousarray(
                img_table[Uc].astype(BF16).reshape(c.ACH, 512, c.KI, c.KIW)
                .transpose(3, 0, 2, 1)),
            "txtRT": np.ascontiguousarray(
                txt_table[Uc].astype(BF16).reshape(c.ACH, 512, c.KT, c.KTW)
                .transpose(3, 0, 2, 1)),
            "embRT": np.ascontiguousarray(
                (emb_table[Uc].T + bcomb).astype(f32)
                .reshape(c.D, c.ACH, 512)),
            "wiN": img_W.astype(BF16), "wtN": txt_W.astype(BF16),
            "wi01": (0.1 * img_W).astype(BF16),
            "wt015": (0.15 * txt_W).astype(BF16),
            "imgbR": img_b.reshape(1, c.D).astype(BF16),
            "txtbR": txt_b.reshape(1, c.D).astype(BF16),
            "rawcat": rawcat,
            "gvW": gate_v_W.astype(f32), "gvB": gate_v_b.reshape(c.D, 1).astype(f32),
            "gtW": gate_t_W.astype(f32), "gtB": gate_t_b.reshape(c.D, 1).astype(f32),
            "q1W": qc_W1.astype(f32), "q1B": qc_b1.reshape(c.D, 1).astype(f32),
            "q2W": qc_W2.astype(f32),
            "Gbd": Gbd.astype(BF16), "GTbd": GTbd.astype(BF16),
            "Mbd16": Mbd.astype(BF16),
            "mkT": np.ascontiguousarray(mk.T).astype(BF16),
            "mindT": np.ascontiguousarray(mind.T).astype(BF16),
            "ind2": ind2.astype(BF16),
            "h0idx": h0x[b0:b1].reshape(c.NPAIR, c.L2, 1),
            "ssidx": ssx[b0:b1].reshape(c.NPAIR, c.L2, 1),
        })
    return in_maps


def run(c: Cfg, trace=False, **inputs):
    nc = _get_program(c)
    in_maps = _prep_inputs(c, **{k: np.asarray(v) for k, v in inputs.items()})
    res = bass_utils.run_bass_kernel_spmd(
        nc, in_maps, core_ids=list(range(c.NC)), trace=trace)
    out = np.concatenate([r["outH"] for r in res.results], axis=0)
    return out.astype(np.float32), res


def kernel(**inputs):
    out, _ = run(REAL, trace=False, **inputs)
    return out
